# revision 17
# baseline (speedup 1.0000x reference)
"""Trainium2 Bass kernel for nn_BoxLoss (YOLO-style box regression loss).

Contract: kernel(**inputs) takes FULL unsharded inputs (numpy), returns the
FULL scalar loss. Pure data parallel over batch across 8 NeuronCores
(4 images per core); each core computes its 12 (scale, image) row losses
on-device and writes a [2,6] partial; the host sums the 96 partials.

The big [B,A,g,g,85] activation tensors are touched ONLY via ONE indirect
(gather) DMA of the 600 matched cells x 4 channels the loss actually reads
(600 descriptors in a single SWDGE instruction).

Single layout end-to-end: partition p = bh*50 + j (image-half, target),
free dim sbl = s*2 + bl (scale, image-parity), channels innermost.

Last-wins dedup without any DRAM roundtrip: the per-target key is spread
to column j via a (j==k) indicator mult, then one PE matmul with a
bh-block indicator broadcasts every target's key to all partitions of its
half; an earlier PE matmul pre-accumulates +16384 onto the k<=j positions
of the same PSUM tile so a plain is_equal only fires for strictly-later
equal keys. The PSUM is integer-rounded before the compare to be robust
to fp32r matmul rounding.

Scheduling: the DVE critical chain (floor -> IoU -> argmax -> cell index)
is kept contiguous; everything else is either on GpSimd or forced (via
explicit deps) to issue after the gather offsets are ready, so the gather
launches as early as possible.
"""

import numpy as np

import concourse.bass as bass
import concourse.bacc as bacc
import concourse.mybir as mybir
import concourse.tile as tile
from concourse.tile import add_dep_helper

NCORES = 8
GRIDS = (52, 26, 13)
A = 3           # anchors per scale
T = 50          # targets per image
PB = 4          # images per core
B_TOTAL = 32
P100 = 2 * T    # partitions: (bh, j)
SBL = 6         # free cols: (s, bl)
SENT = 8112.0   # > max cell id (3*52*52 - 1)
GATHER_SPLIT = 6
USE_MOD = False
USE_DIV = False

F32 = mybir.dt.float32
I32 = mybir.dt.int32

_SCALE_ELEMS = [PB * A * g * g * 85 for g in GRIDS]
_SCALE_BASE = [0, _SCALE_ELEMS[0], _SCALE_ELEMS[0] + _SCALE_ELEMS[1]]
OUTCAT_ELEMS = sum(_SCALE_ELEMS)

# hostpack column layout ([100, 146])
_H_T4 = 0       # [0,24)    t4 = raw_target * g  (sbl, c)
_H_WH2 = 24     # [24,48)   [-twh/2 | +twh/2]  (h, sbl, q)
_H_NAWH = 48    # [48,84)   -anchor/2 (q, sbl, a)
_H_AWH = 84     # [84,120)  +anchor/2 (q, sbl, a)
_H_AREAA = 120  # [120,138) anchor area (sbl, a)
_H_G6 = 138     # [138,144) g per sbl
_H_C85 = 144    # [144,150) constant 85.0
_H_BG = 150     # [150,156) base + b*A*85*g^2 (per-partition: bh)
_H_HW85 = 156   # [156,162) 85*g^2
_H_HW6 = 162    # [162,168) g^2
_H_ONESU = 168  # [168,170) half-indicator columns
_H_VAL = 170    # [170,172) valid mask per bl
_HP_TOT = 172

# consts column layout ([100, 500])
_C_J = 0        # [0,300)   J300[q, sbl*50+k] = (j(q) == k)
_C_TRI = 300    # [300,400) 16384 * (bh(q)==bh(p)) * (j(q) <= j(p))
_C_BH = 400     # [400,500) (bh(q) == bh(p))
_C_TOT = 500


def _host_consts():
    sbl = np.arange(SBL)
    s = sbl // 2
    g = np.array(GRIDS, dtype=np.float64)[s]              # [6]

    p = np.arange(P100)
    bh = p // T
    base = np.array(_SCALE_BASE, dtype=np.float64)[s][None, :]
    b = (2 * bh[:, None] + (sbl % 2)[None, :])
    bg = base + b * (A * 85) * (g ** 2)[None, :]          # [100, 6]

    hp = np.zeros((P100, _HP_TOT), np.float64)
    hp[:, _H_G6:_H_G6 + 6] = g[None, :]
    hp[:, _H_C85:_H_C85 + 6] = 85.0
    hp[:, _H_BG:_H_BG + 6] = bg
    hp[:, _H_HW85:_H_HW85 + 6] = 85.0 * (g ** 2)[None, :]
    hp[:, _H_HW6:_H_HW6 + 6] = (g ** 2)[None, :]
    hp[p, _H_ONESU + bh] = 1.0
    return hp.astype(np.float32)


def _inline_consts():
    p = np.arange(P100)
    j = p % T
    bh = p // T
    cst = np.zeros((P100, _C_TOT), np.float32)
    k6 = np.tile(np.arange(T), SBL)[None, :]
    cst[:, _C_J:_C_J + 300] = (j[:, None] == k6).astype(np.float32)
    same_bh = bh[:, None] == bh[None, :]
    cst[:, _C_TRI:_C_TRI + 100] = 16384.0 * (
        same_bh & (j[:, None] <= j[None, :])).astype(np.float32)
    cst[:, _C_BH:_C_BH + 100] = same_bh.astype(np.float32)
    return np.ascontiguousarray(cst)


def build_nc(use_collective: bool = False):
    nc = bacc.Bacc("TRN2", target_bir_lowering=False, debug=False,
                   num_devices=NCORES)

    hp_d = nc.dram_tensor("hostpack", [P100, _HP_TOT], F32, kind="ExternalInput")
    outcat_d = nc.dram_tensor("outcat", [OUTCAT_ELEMS], F32, kind="ExternalInput")
    loss_d = nc.dram_tensor("loss", [P100, 24], F32, kind="ExternalOutput")
    cst_d = nc.inline_tensor(_inline_consts(), name="cst")
    bhsel_np = np.zeros((2, P100), np.float32)
    bhsel_np[0, 0:T] = 1.0
    bhsel_np[1, T:P100] = 1.0
    bhsel_d = nc.inline_tensor(bhsel_np, name="bhsel")

    AL = mybir.AluOpType
    AX = mybir.AxisListType.X

    with tile.TileContext(nc) as tc:
        with (
            tc.tile_pool(name="sbuf", bufs=1) as sp,
            tc.tile_pool(name="psum", bufs=1, space="PSUM") as pp,
        ):
            def tt(out, in0, in1, op):
                return nc.vector.tensor_tensor(out=out, in0=in0, in1=in1, op=op)

            def gtt(out, in0, in1, op):
                return nc.gpsimd.tensor_tensor(out=out, in0=in0, in1=in1, op=op)

            def ts(out, in0, s1, op, s2=None, op2=None):
                if op2 is None:
                    return nc.vector.tensor_scalar(out=out, in0=in0, scalar1=s1,
                                                   scalar2=None, op0=op)
                return nc.vector.tensor_scalar(out=out, in0=in0, scalar1=s1,
                                               scalar2=s2, op0=op, op1=op2)

            def stt(out, in0, scalar, in1, op0, op1):
                return nc.vector.scalar_tensor_tensor(
                    out=out, in0=in0, scalar=scalar, in1=in1, op0=op0, op1=op1)

            def act_rsqrt(out, in_):
                eng = nc.scalar
                bias = nc.const_aps.scalar_like(0.0, in_)
                return eng.add_instruction(
                    mybir.InstActivation(
                        name=nc.get_next_instruction_name(),
                        func=mybir.ActivationFunctionType.Rsqrt,
                        ins=[eng.lower_ap(in_), eng.lower_ap(bias),
                             mybir.ImmediateValue(dtype=mybir.dt.float32,
                                                  value=1.0),
                             mybir.ImmediateValue(dtype=mybir.dt.float32,
                                                  value=0.0)],
                        outs=[eng.lower_ap(out)]))

            _tn = [0]

            def new(shape, dt=F32):
                _tn[0] += 1
                return sp.tile(shape, dt, name=f"t{_tn[0]}")

            # ---------- input loads ----------
            hp = new([P100, _HP_TOT])
            nc.sync.dma_start(out=hp[:], in_=hp_d[:, :])
            cst = new([P100, _C_TOT])
            nc.scalar.dma_start(out=cst[:], in_=cst_d[:, :])
            bhsel = new([2, P100])
            nc.scalar.dma_start(out=bhsel[:], in_=bhsel_d[:, :])

            def C(c0, w):
                return hp[:, c0:c0 + w]

            J300v = cst[:, _C_J:_C_J + 300].rearrange(
                "p (sbl k) -> p sbl k", k=T)
            onesU = C(_H_ONESU, 2)

            # PSUM pre-accumulate: latNeg[p,(sbl,k)] = 16384*(k<=j(p))
            psK = pp.tile([P100, 300], F32, name="psK")
            nc.tensor.matmul(out=psK[:], lhsT=cst[:, _C_TRI:_C_TRI + 100],
                             rhs=cst[:, _C_J:_C_J + 300],
                             start=True, stop=False)

            # ---------- critical DVE chain to gather offsets ----------
            t4 = C(_H_T4, 24)
            t4v = t4.rearrange("p (sbl c) -> p sbl c", c=4)
            txy = t4v[:, :, 0:2]
            twh = t4v[:, :, 2:4]

            frac = new([P100, 12])
            if USE_MOD:
                ts(frac[:], txy, 1.0, AL.mod)
                fxy = new([P100, 12])
                tt(fxy[:], txy, frac[:], AL.subtract)
                zt05 = new([P100, 12])
                ts(zt05[:], frac[:], -0.5, AL.add)
            else:
                r2 = new([P100, 12])
                ts(r2[:], txy, float(2 ** 23), AL.add, -float(2 ** 23), AL.add)
                gtm = new([P100, 12])
                tt(gtm[:], r2[:], txy, AL.is_gt)
                fxy = new([P100, 12])
                tt(fxy[:], r2[:], gtm[:], AL.subtract)
                zt05 = new([P100, 12])
                stt(zt05[:], txy, -0.5, fxy[:], AL.add, AL.subtract)
            fv = fxy[:].rearrange("p (sbl q) -> p sbl q", q=2)
            cx = fv[:, :, 0:1]
            cy = fv[:, :, 1:2]

            LOHI = new([P100, 24])
            tt(LOHI[:].rearrange("p (h q) -> p h q", q=12),
               C(_H_WH2, 24).rearrange("p (h q) -> p h q", q=12),
               zt05[:, None, :].to_broadcast([P100, 2, 12]), AL.add)

            def bcQ(t12):
                return (t12.rearrange("p (sbl q) -> p q sbl", q=2)
                        [:, :, :, None].to_broadcast([P100, 2, SBL, 3]))

            P0 = new([P100, 36])
            tt(P0[:], bcQ(LOHI[:, 0:12]), C(_H_NAWH, 36), AL.max)
            P1 = new([P100, 36])
            tt(P1[:], bcQ(LOHI[:, 12:24]), C(_H_AWH, 36), AL.min)
            D = new([P100, 36]); tt(D[:], P1[:], P0[:], AL.subtract)
            M0 = new([P100, 36]); ts(M0[:], D[:], 0.0, AL.max)
            inter = new([P100, 18])
            tt(inter[:], M0[:, 0:18], M0[:, 18:36], AL.mult)

            # GpSimd side branch: union pre-sum and scaled cell column index
            areat = new([P100, 6])
            gtt(areat[:], t4v[:, :, 2:3], t4v[:, :, 3:4], AL.mult)
            un1 = new([P100, 18])
            gtt(un1[:], areat[:, :, None].to_broadcast([P100, SBL, 3]),
                C(_H_AREAA, 18), AL.add)
            cyg = new([P100, 6])
            gtt(cyg[:], cy, C(_H_G6, 6), AL.mult)
            cbc = new([P100, 6])
            gtt(cbc[:], cyg[:], cx, AL.add)
            cbca = new([P100, 6])
            gtt(cbca[:], cbc[:], C(_H_C85, 6), AL.mult)
            cbc85 = new([P100, 6])
            gtt(cbc85[:], cbca[:], C(_H_BG, 6), AL.add)

            union = new([P100, 18]); tt(union[:], un1[:], inter[:], AL.subtract)
            iou = new([P100, 18])
            if USE_DIV:
                tt(iou[:], inter[:], union[:], AL.divide)
            else:
                runi = new([P100, 18])
                nc.vector.reciprocal(out=runi[:], in_=union[:])
                tt(iou[:], inter[:], runi[:], AL.mult)
            iv = iou[:].rearrange("p (sbl a) -> p sbl a", a=3)
            overlap = new([P100, 6])
            nc.vector.reduce_max(out=overlap[:], in_=iv, axis=AX)
            nn01 = new([P100, 12])
            tt(nn01[:], iv[:, :, 0:2],
               overlap[:, :, None].to_broadcast([P100, SBL, 2]), AL.is_lt)
            nv = nn01[:].rearrange("p (sbl e) -> p sbl e", e=2)
            anc = new([P100, 6])
            stt(anc[:], nv[:, :, 1:2], 1.0, nv[:, :, 0:1], AL.add, AL.mult)
            a85 = new([P100, 6]); tt(a85[:], anc[:], C(_H_HW85, 6), AL.mult)
            idxi = new([P100, 6], I32)
            idxi_i = tt(idxi[:], a85[:], cbc85[:], AL.add)

            # ---------- 6 gathers (3 pair tiles; adjacent DMAs hit
            # distinct tiles so the SWDGE queue pipelines) ----------
            gpair = [new([P100, 8]) for _ in range(3)]
            for q in (0, 2, 4, 1, 3, 5):
                s_, bl = q // 2, q % 2
                nc.gpsimd.indirect_dma_start(
                    out=gpair[s_][:, bl * 4:(bl + 1) * 4], out_offset=None,
                    in_=outcat_d[:].unsqueeze(1),
                    in_offset=bass.IndirectOffsetOnAxis(ap=idxi[:, q:q + 1],
                                                        axis=0),
                )

            def after_idxi(op):
                add_dep_helper(op.ins, idxi_i.ins, True,
                               "keep the pre-gather DVE chain contiguous")
                return op

            # ---------- off-critical DVE work while the gather flies ----
            rstw = new([P100, 12])
            act_rsqrt(rstw[:], twh)

            om = new([P100, 6])
            after_idxi(ts(om[:], overlap[:], 0.5, AL.is_gt))
            m = new([P100, 6])
            tt(m[:], om[:].rearrange("p (s bl) -> p s bl", bl=2),
               C(_H_VAL, 2)[:, None, :].to_broadcast([P100, 3, 2]), AL.mult)
            ahw6 = new([P100, 6])
            after_idxi(tt(ahw6[:], anc[:], C(_H_HW6, 6), AL.mult))
            cell = new([P100, 6]); tt(cell[:], ahw6[:], cbc[:], AL.add)
            kk = new([P100, 6])
            stt(kk[:], cell[:], -SENT, m[:], AL.add, AL.mult)

            # dedup: spread key to col j, PE-broadcast to the whole half
            rhsE = new([P100, 300])
            tt(rhsE[:].rearrange("p (sbl k) -> p sbl k", k=T),
               kk[:, :, None].to_broadcast([P100, SBL, T]), J300v, AL.mult)
            nc.tensor.matmul(out=psK[:], lhsT=cst[:, _C_BH:_C_BH + 100],
                             rhs=rhsE[:], start=False, stop=True)
            # round PSUM to integers: robust to fp32r rounding on HW
            psKr = new([P100, 300])
            ts(psKr[:], psK[:], float(2 ** 23), AL.add, -float(2 ** 23), AL.add)
            E = new([P100, 300])
            tt(E[:].rearrange("p (sbl k) -> p sbl k", k=T),
               kk[:, :, None].to_broadcast([P100, SBL, T]),
               psKr[:].rearrange("p (sbl k) -> p sbl k", k=T), AL.is_equal)
            ov = new([P100, 6])
            nc.vector.reduce_max(out=ov[:],
                                 in_=E[:].rearrange("p (sbl k) -> p sbl k", k=T),
                                 axis=AX)
            W0 = new([P100, 6])
            stt(W0[:], ov[:], 0.0, m[:], AL.is_equal, AL.mult)

            # per-slot coefficient co2 = W0 / (2*B*max(n,1)), ready
            # before the gathers land: counts via PE, denominators on
            # [2,*], PE-broadcast back to all partitions
            M1c = pp.tile([2, 6], F32, name="M1c")
            nc.tensor.matmul(out=M1c[:], lhsT=onesU, rhs=W0[:],
                             start=True, stop=True)
            mx2 = new([2, 6])
            ts(mx2[:], M1c[:], 1.0, AL.max, 2.0 * B_TOTAL, AL.mult)
            rden = new([2, 6]); nc.vector.reciprocal(out=rden[:], in_=mx2[:])
            psB = pp.tile([P100, 6], F32, name="psB")
            nc.tensor.matmul(out=psB[:], lhsT=bhsel[:], rhs=rden[:],
                             start=True, stop=True)
            co2 = new([P100, 6])
            co2_i = tt(co2[:], W0[:], psB[:], AL.mult)

            # ---------- per-pair stripe math on gathered preds ----------
            SQ = new([P100, 24])
            for s_ in range(3):
                g8 = gpair[s_]
                gvs = g8[:].rearrange("p (bl c) -> p bl c", c=4)
                rspw = new([P100, 4])
                act_rsqrt(rspw[:], gvs[:, :, 2:4])
                sel = new([P100, 8])
                selv = sel[:].rearrange("p (bl c) -> p bl c", c=4)
                sx = tt(selv[:, :, 0:2], gvs[:, :, 0:2],
                        t4v[:, 2 * s_:2 * s_ + 2, 0:2], AL.subtract)
                if s_ == 0:
                    # keep the whole dedup/coefficient path ahead of any
                    # stripe work in the DVE stream
                    add_dep_helper(sx.ins, co2_i.ins, True,
                                   "stripes issue after the dedup tail")
                tt(selv[:, :, 2:4],
                   rspw[:].rearrange("p (bl q) -> p bl q", q=2),
                   rstw[:].rearrange("p (sbl q) -> p sbl q", q=2)
                   [:, 2 * s_:2 * s_ + 2, :], AL.subtract)
                selc = new([P100, 8])
                tt(selc[:].rearrange("p (bl c) -> p bl c", c=4), selv,
                   co2[:, 2 * s_:2 * s_ + 2, None].to_broadcast(
                       [P100, 2, 4]), AL.mult)
                tt(SQ[:, 8 * s_:8 * s_ + 8].rearrange(
                       "p (bl c) -> p bl c", c=4),
                   selc[:].rearrange("p (bl c) -> p bl c", c=4),
                   selv, AL.mult)
            nc.sync.dma_start(out=loss_d[:, :], in_=SQ[:])

    nc.compile()
    return nc


_HOST_CONSTS = _host_consts()
_G24 = np.array([GRIDS[s // 2] for s in range(SBL)], np.float32)


def make_in_maps(output0, anchors0, output1, anchors1, output2, anchors2,
                 targets):
    outs = [np.asarray(output0), np.asarray(output1), np.asarray(output2)]
    ancs = [np.asarray(anchors0), np.asarray(anchors1), np.asarray(anchors2)]
    tg = np.asarray(targets)

    nawh = np.zeros(36, np.float32)
    awh = np.zeros(36, np.float32)
    areaa = np.zeros(18, np.float32)
    for s_ in range(3):
        for a_ in range(A):
            aw, ah = float(ancs[s_][a_, 0]), float(ancs[s_][a_, 1])
            for bl in range(2):
                sbl = s_ * 2 + bl
                for q_, dim in ((0, aw), (1, ah)):
                    nawh[q_ * 18 + sbl * 3 + a_] = -0.5 * dim
                    awh[q_ * 18 + sbl * 3 + a_] = 0.5 * dim
                areaa[sbl * 3 + a_] = aw * ah

    in_maps = []
    for c in range(NCORES):
        sl = slice(c * PB, (c + 1) * PB)
        raw = tg[sl, :, 1:5].astype(np.float32)          # [4, 50, 4]
        tg8 = (raw.reshape(2, 2, T, 4)                    # (bh, bl, j, c)
               .transpose(0, 2, 1, 3).reshape(P100, 2, 4))  # (bh,j) x bl x c
        t4 = (tg8[:, None, :, :] *
              _G24.reshape(1, 3, 2, 1)[:, :, :, [0, 0, 0, 0]])  # wrong shape fix below
        # t4[p, (s, bl), c] = raw * g(s)
        t4 = (tg8[:, None, :, :] * _G24.reshape(3, 2)[None, :, :, None]
              ).reshape(P100, 24)
        t4r = t4.reshape(P100, 6, 4)
        wh2 = np.concatenate([-0.5 * t4r[:, :, 2:4].reshape(P100, 12),
                              0.5 * t4r[:, :, 2:4].reshape(P100, 12)], axis=1)
        valid = (np.abs(raw).sum(2) > 0).astype(np.float32)  # [4, 50]
        val2 = (valid.reshape(2, 2, T).transpose(0, 2, 1)
                .reshape(P100, 2))                           # (bh,j) x bl
        hostpack = _HOST_CONSTS.copy()
        hostpack[:, _H_T4:_H_T4 + 24] = t4
        hostpack[:, _H_WH2:_H_WH2 + 24] = wh2
        hostpack[:, _H_VAL:_H_VAL + 2] = val2
        hostpack[:, _H_NAWH:_H_NAWH + 36] = nawh[None, :]
        hostpack[:, _H_AWH:_H_AWH + 36] = awh[None, :]
        hostpack[:, _H_AREAA:_H_AREAA + 18] = areaa[None, :]
        outcat = np.concatenate([o[sl].ravel() for o in outs]).astype(np.float32)
        in_maps.append({"hostpack": np.ascontiguousarray(hostpack),
                        "outcat": outcat})
    return in_maps


_NC_CACHE = {}


def kernel(output0, anchors0, output1, anchors1, output2, anchors2, targets):
    import time
    from concourse.bass_utils import run_bass_kernel_spmd

    if "nc" not in _NC_CACHE:
        _NC_CACHE["nc"] = build_nc(use_collective=False)
    nc = _NC_CACHE["nc"]
    in_maps = make_in_maps(output0, anchors0, output1, anchors1, output2,
                           anchors2, targets)
    res = None
    for attempt in range(3):
        try:
            res = run_bass_kernel_spmd(nc, in_maps, list(range(NCORES)))
            break
        except Exception:
            # transient NRT device errors have been observed; back off + retry
            if attempt == 2:
                raise
            time.sleep(20.0 * (attempt + 1))
    total = np.float64(0.0)
    for c in range(NCORES):
        total += np.asarray(res.results[c]["loss"], dtype=np.float64).sum()
    return np.float32(total)


# revision 18
# speedup vs baseline: 1.0964x; 1.0964x over previous
"""Trainium2 Bass kernel for nn_BoxLoss (YOLO-style box regression loss).

Contract: kernel(**inputs) takes FULL unsharded inputs (numpy), returns the
FULL scalar loss. Pure data parallel over batch across 8 NeuronCores
(4 images per core); each core computes its 12 (scale, image) row losses
on-device and writes a [2,6] partial; the host sums the 96 partials.

The big [B,A,g,g,85] activation tensors are touched ONLY via ONE indirect
(gather) DMA of the 600 matched cells x 4 channels the loss actually reads
(600 descriptors in a single SWDGE instruction).

Single layout end-to-end: partition p = bh*50 + j (image-half, target),
free dim sbl = s*2 + bl (scale, image-parity), channels innermost.

Last-wins dedup without any DRAM roundtrip: the per-target key is spread
to column j via a (j==k) indicator mult, then one PE matmul with a
bh-block indicator broadcasts every target's key to all partitions of its
half; an earlier PE matmul pre-accumulates +16384 onto the k<=j positions
of the same PSUM tile so a plain is_equal only fires for strictly-later
equal keys. The PSUM is integer-rounded before the compare to be robust
to fp32r matmul rounding.

Scheduling: the DVE critical chain (floor -> IoU -> argmax -> cell index)
is kept contiguous; everything else is either on GpSimd or forced (via
explicit deps) to issue after the gather offsets are ready, so the gather
launches as early as possible.
"""

import numpy as np

import concourse.bass as bass
import concourse.bacc as bacc
import concourse.mybir as mybir
import concourse.tile as tile
from concourse.tile import add_dep_helper

NCORES = 8
GRIDS = (52, 26, 13)
A = 3           # anchors per scale
T = 50          # targets per image
PB = 4          # images per core
B_TOTAL = 32
P100 = 2 * T    # partitions: (bh, j)
SBL = 6         # free cols: (s, bl)
SENT = 8112.0   # > max cell id (3*52*52 - 1)
GATHER_SPLIT = 6
USE_MOD = False
USE_DIV = False

F32 = mybir.dt.float32
I32 = mybir.dt.int32

_SCALE_ELEMS = [PB * A * g * g * 85 for g in GRIDS]
_SCALE_BASE = [0, _SCALE_ELEMS[0], _SCALE_ELEMS[0] + _SCALE_ELEMS[1]]
OUTCAT_ELEMS = sum(_SCALE_ELEMS)

# hostpack column layout ([100, 146])
_H_T4 = 0       # [0,24)    t4 = raw_target * g  (sbl, c)
_H_WH2 = 24     # [24,48)   [-twh/2 | +twh/2]  (h, sbl, q)
_H_NAWH = 48    # [48,84)   -anchor/2 (q, sbl, a)
_H_AWH = 84     # [84,120)  +anchor/2 (q, sbl, a)
_H_AREAA = 120  # [120,138) anchor area (sbl, a)
_H_G6 = 138     # [138,144) g per sbl
_H_C85 = 144    # [144,150) constant 85.0
_H_BG = 150     # [150,156) base + b*A*85*g^2 (per-partition: bh)
_H_HW85 = 156   # [156,162) 85*g^2
_H_HW6 = 162    # [162,168) g^2
_H_ONESU = 168  # [168,170) half-indicator columns
_H_VAL = 170    # [170,172) valid mask per bl
_HP_TOT = 172

# consts column layout ([100, 500])
_C_J = 0        # [0,300)   J300[q, sbl*50+k] = (j(q) == k)
_C_TRI = 300    # [300,400) 16384 * (bh(q)==bh(p)) * (j(q) <= j(p))
_C_BH = 400     # [400,500) (bh(q) == bh(p))
_C_TOT = 500


def _host_consts():
    sbl = np.arange(SBL)
    s = sbl // 2
    g = np.array(GRIDS, dtype=np.float64)[s]              # [6]

    p = np.arange(P100)
    bh = p // T
    base = np.array(_SCALE_BASE, dtype=np.float64)[s][None, :]
    b = (2 * bh[:, None] + (sbl % 2)[None, :])
    bg = base + b * (A * 85) * (g ** 2)[None, :]          # [100, 6]

    hp = np.zeros((P100, _HP_TOT), np.float64)
    hp[:, _H_G6:_H_G6 + 6] = g[None, :]
    hp[:, _H_C85:_H_C85 + 6] = 85.0
    hp[:, _H_BG:_H_BG + 6] = bg
    hp[:, _H_HW85:_H_HW85 + 6] = 85.0 * (g ** 2)[None, :]
    hp[:, _H_HW6:_H_HW6 + 6] = (g ** 2)[None, :]
    hp[p, _H_ONESU + bh] = 1.0
    return hp.astype(np.float32)


def _inline_consts():
    p = np.arange(P100)
    j = p % T
    bh = p // T
    cst = np.zeros((P100, _C_TOT), np.float32)
    k6 = np.tile(np.arange(T), SBL)[None, :]
    cst[:, _C_J:_C_J + 300] = (j[:, None] == k6).astype(np.float32)
    same_bh = bh[:, None] == bh[None, :]
    cst[:, _C_TRI:_C_TRI + 100] = 16384.0 * (
        same_bh & (j[:, None] <= j[None, :])).astype(np.float32)
    cst[:, _C_BH:_C_BH + 100] = same_bh.astype(np.float32)
    return np.ascontiguousarray(cst)


def build_nc(use_collective: bool = False):
    nc = bacc.Bacc("TRN2", target_bir_lowering=False, debug=False,
                   num_devices=NCORES)

    hp_d = nc.dram_tensor("hostpack", [P100, _HP_TOT], F32, kind="ExternalInput")
    outcat_d = nc.dram_tensor("outcat", [OUTCAT_ELEMS], F32, kind="ExternalInput")
    loss_d = nc.dram_tensor("loss", [P100, 24], F32, kind="ExternalOutput")
    cst_d = nc.inline_tensor(_inline_consts(), name="cst")
    bhsel_np = np.zeros((2, P100), np.float32)
    bhsel_np[0, 0:T] = 1.0
    bhsel_np[1, T:P100] = 1.0
    bhsel_d = nc.inline_tensor(bhsel_np, name="bhsel")

    AL = mybir.AluOpType
    AX = mybir.AxisListType.X

    with tile.TileContext(nc) as tc:
        with (
            tc.tile_pool(name="sbuf", bufs=1) as sp,
            tc.tile_pool(name="psum", bufs=1, space="PSUM") as pp,
        ):
            def tt(out, in0, in1, op):
                return nc.vector.tensor_tensor(out=out, in0=in0, in1=in1, op=op)

            def gtt(out, in0, in1, op):
                return nc.gpsimd.tensor_tensor(out=out, in0=in0, in1=in1, op=op)

            def ts(out, in0, s1, op, s2=None, op2=None):
                if op2 is None:
                    return nc.vector.tensor_scalar(out=out, in0=in0, scalar1=s1,
                                                   scalar2=None, op0=op)
                return nc.vector.tensor_scalar(out=out, in0=in0, scalar1=s1,
                                               scalar2=s2, op0=op, op1=op2)

            def stt(out, in0, scalar, in1, op0, op1):
                return nc.vector.scalar_tensor_tensor(
                    out=out, in0=in0, scalar=scalar, in1=in1, op0=op0, op1=op1)

            _tn = [0]

            def new(shape, dt=F32):
                _tn[0] += 1
                return sp.tile(shape, dt, name=f"t{_tn[0]}")

            # ---------- input loads ----------
            hp = new([P100, _HP_TOT])
            nc.sync.dma_start(out=hp[:], in_=hp_d[:, :])
            cst = new([P100, _C_TOT])
            nc.scalar.dma_start(out=cst[:], in_=cst_d[:, :])
            bhsel = new([2, P100])
            nc.scalar.dma_start(out=bhsel[:], in_=bhsel_d[:, :])

            def C(c0, w):
                return hp[:, c0:c0 + w]

            J300v = cst[:, _C_J:_C_J + 300].rearrange(
                "p (sbl k) -> p sbl k", k=T)
            onesU = C(_H_ONESU, 2)

            # PSUM pre-accumulate: latNeg[p,(sbl,k)] = 16384*(k<=j(p))
            psK = pp.tile([P100, 300], F32, name="psK")
            nc.tensor.matmul(out=psK[:], lhsT=cst[:, _C_TRI:_C_TRI + 100],
                             rhs=cst[:, _C_J:_C_J + 300],
                             start=True, stop=False)

            # ---------- critical DVE chain to gather offsets ----------
            t4 = C(_H_T4, 24)
            t4v = t4.rearrange("p (sbl c) -> p sbl c", c=4)
            txy = t4v[:, :, 0:2]
            twh = t4v[:, :, 2:4]

            frac = new([P100, 12])
            if USE_MOD:
                ts(frac[:], txy, 1.0, AL.mod)
                fxy = new([P100, 12])
                tt(fxy[:], txy, frac[:], AL.subtract)
                zt05 = new([P100, 12])
                ts(zt05[:], frac[:], -0.5, AL.add)
            else:
                r2 = new([P100, 12])
                ts(r2[:], txy, float(2 ** 23), AL.add, -float(2 ** 23), AL.add)
                gtm = new([P100, 12])
                tt(gtm[:], r2[:], txy, AL.is_gt)
                fxy = new([P100, 12])
                tt(fxy[:], r2[:], gtm[:], AL.subtract)
                zt05 = new([P100, 12])
                stt(zt05[:], txy, -0.5, fxy[:], AL.add, AL.subtract)
            fv = fxy[:].rearrange("p (sbl q) -> p sbl q", q=2)
            cx = fv[:, :, 0:1]
            cy = fv[:, :, 1:2]

            LOHI = new([P100, 24])
            tt(LOHI[:].rearrange("p (h q) -> p h q", q=12),
               C(_H_WH2, 24).rearrange("p (h q) -> p h q", q=12),
               zt05[:, None, :].to_broadcast([P100, 2, 12]), AL.add)

            def bcQ(t12):
                return (t12.rearrange("p (sbl q) -> p q sbl", q=2)
                        [:, :, :, None].to_broadcast([P100, 2, SBL, 3]))

            P0 = new([P100, 36])
            tt(P0[:], bcQ(LOHI[:, 0:12]), C(_H_NAWH, 36), AL.max)
            P1 = new([P100, 36])
            tt(P1[:], bcQ(LOHI[:, 12:24]), C(_H_AWH, 36), AL.min)
            D = new([P100, 36]); tt(D[:], P1[:], P0[:], AL.subtract)
            M0 = new([P100, 36]); ts(M0[:], D[:], 0.0, AL.max)
            inter = new([P100, 18])
            tt(inter[:], M0[:, 0:18], M0[:, 18:36], AL.mult)

            # GpSimd side branch: union pre-sum and scaled cell column index
            areat = new([P100, 6])
            gtt(areat[:], t4v[:, :, 2:3], t4v[:, :, 3:4], AL.mult)
            un1 = new([P100, 18])
            gtt(un1[:], areat[:, :, None].to_broadcast([P100, SBL, 3]),
                C(_H_AREAA, 18), AL.add)
            cyg = new([P100, 6])
            gtt(cyg[:], cy, C(_H_G6, 6), AL.mult)
            cbc = new([P100, 6])
            gtt(cbc[:], cyg[:], cx, AL.add)
            cbca = new([P100, 6])
            gtt(cbca[:], cbc[:], C(_H_C85, 6), AL.mult)
            cbc85 = new([P100, 6])
            gtt(cbc85[:], cbca[:], C(_H_BG, 6), AL.add)

            union = new([P100, 18]); tt(union[:], un1[:], inter[:], AL.subtract)
            iou = new([P100, 18])
            if USE_DIV:
                tt(iou[:], inter[:], union[:], AL.divide)
            else:
                runi = new([P100, 18])
                nc.vector.reciprocal(out=runi[:], in_=union[:])
                tt(iou[:], inter[:], runi[:], AL.mult)
            iv = iou[:].rearrange("p (sbl a) -> p sbl a", a=3)
            overlap = new([P100, 6])
            nc.vector.reduce_max(out=overlap[:], in_=iv, axis=AX)
            nn01 = new([P100, 12])
            tt(nn01[:], iv[:, :, 0:2],
               overlap[:, :, None].to_broadcast([P100, SBL, 2]), AL.is_lt)
            nv = nn01[:].rearrange("p (sbl e) -> p sbl e", e=2)
            anc = new([P100, 6])
            stt(anc[:], nv[:, :, 1:2], 1.0, nv[:, :, 0:1], AL.add, AL.mult)
            a85 = new([P100, 6]); tt(a85[:], anc[:], C(_H_HW85, 6), AL.mult)
            idxi = new([P100, 6], I32)
            idxi_i = tt(idxi[:], a85[:], cbc85[:], AL.add)

            # ---------- 6 gathers (3 pair tiles; adjacent DMAs hit
            # distinct tiles so the SWDGE queue pipelines) ----------
            gpair = [new([P100, 8]) for _ in range(3)]
            for q in (0, 2, 4, 1, 3, 5):
                s_, bl = q // 2, q % 2
                nc.gpsimd.indirect_dma_start(
                    out=gpair[s_][:, bl * 4:(bl + 1) * 4], out_offset=None,
                    in_=outcat_d[:].unsqueeze(1),
                    in_offset=bass.IndirectOffsetOnAxis(ap=idxi[:, q:q + 1],
                                                        axis=0),
                )

            def after_idxi(op):
                add_dep_helper(op.ins, idxi_i.ins, True,
                               "keep the pre-gather DVE chain contiguous")
                return op

            # ---------- off-critical DVE work while the gather flies ----
            rtwh = new([P100, 12])
            after_idxi(nc.vector.reciprocal(out=rtwh[:], in_=twh))
            rstw = new([P100, 12]); nc.scalar.sqrt(out=rstw[:], in_=rtwh[:])

            om = new([P100, 6])
            after_idxi(ts(om[:], overlap[:], 0.5, AL.is_gt))
            m = new([P100, 6])
            tt(m[:], om[:].rearrange("p (s bl) -> p s bl", bl=2),
               C(_H_VAL, 2)[:, None, :].to_broadcast([P100, 3, 2]), AL.mult)
            ahw6 = new([P100, 6])
            after_idxi(tt(ahw6[:], anc[:], C(_H_HW6, 6), AL.mult))
            cell = new([P100, 6]); tt(cell[:], ahw6[:], cbc[:], AL.add)
            kk = new([P100, 6])
            stt(kk[:], cell[:], -SENT, m[:], AL.add, AL.mult)

            # dedup: spread key to col j, PE-broadcast to the whole half
            rhsE = new([P100, 300])
            tt(rhsE[:].rearrange("p (sbl k) -> p sbl k", k=T),
               kk[:, :, None].to_broadcast([P100, SBL, T]), J300v, AL.mult)
            nc.tensor.matmul(out=psK[:], lhsT=cst[:, _C_BH:_C_BH + 100],
                             rhs=rhsE[:], start=False, stop=True)
            # round PSUM to integers: robust to fp32r rounding on HW
            psKr = new([P100, 300])
            ts(psKr[:], psK[:], float(2 ** 23), AL.add, -float(2 ** 23), AL.add)
            E = new([P100, 300])
            tt(E[:].rearrange("p (sbl k) -> p sbl k", k=T),
               kk[:, :, None].to_broadcast([P100, SBL, T]),
               psKr[:].rearrange("p (sbl k) -> p sbl k", k=T), AL.is_equal)
            ov = new([P100, 6])
            nc.vector.reduce_max(out=ov[:],
                                 in_=E[:].rearrange("p (sbl k) -> p sbl k", k=T),
                                 axis=AX)
            W0 = new([P100, 6])
            stt(W0[:], ov[:], 0.0, m[:], AL.is_equal, AL.mult)

            # per-slot coefficient co2 = W0 / (2*B*max(n,1)), ready
            # before the gathers land: counts via PE, denominators on
            # [2,*], PE-broadcast back to all partitions
            M1c = pp.tile([2, 6], F32, name="M1c")
            nc.tensor.matmul(out=M1c[:], lhsT=onesU, rhs=W0[:],
                             start=True, stop=True)
            mx2 = new([2, 6])
            ts(mx2[:], M1c[:], 1.0, AL.max, 2.0 * B_TOTAL, AL.mult)
            rden = new([2, 6]); nc.vector.reciprocal(out=rden[:], in_=mx2[:])
            psB = pp.tile([P100, 6], F32, name="psB")
            nc.tensor.matmul(out=psB[:], lhsT=bhsel[:], rhs=rden[:],
                             start=True, stop=True)
            co2 = new([P100, 6])
            co2_i = tt(co2[:], W0[:], psB[:], AL.mult)

            # ---------- per-pair stripe math on gathered preds ----------
            SQ = new([P100, 24])
            for s_ in range(3):
                g8 = gpair[s_]
                gvs = g8[:].rearrange("p (bl c) -> p bl c", c=4)
                rcpw = new([P100, 4])
                ri = nc.vector.reciprocal(out=rcpw[:], in_=gvs[:, :, 2:4])
                if s_ == 0:
                    # keep the whole dedup/coefficient path ahead of any
                    # stripe work in the DVE stream
                    add_dep_helper(ri.ins, co2_i.ins, True,
                                   "stripes issue after the dedup tail")
                rspw = new([P100, 4]); nc.scalar.sqrt(out=rspw[:], in_=rcpw[:])
                sel = new([P100, 8])
                selv = sel[:].rearrange("p (bl c) -> p bl c", c=4)
                tt(selv[:, :, 0:2], gvs[:, :, 0:2],
                   t4v[:, 2 * s_:2 * s_ + 2, 0:2], AL.subtract)
                tt(selv[:, :, 2:4],
                   rspw[:].rearrange("p (bl q) -> p bl q", q=2),
                   rstw[:].rearrange("p (sbl q) -> p sbl q", q=2)
                   [:, 2 * s_:2 * s_ + 2, :], AL.subtract)
                selc = new([P100, 8])
                tt(selc[:].rearrange("p (bl c) -> p bl c", c=4), selv,
                   co2[:, 2 * s_:2 * s_ + 2, None].to_broadcast(
                       [P100, 2, 4]), AL.mult)
                tt(SQ[:, 8 * s_:8 * s_ + 8].rearrange(
                       "p (bl c) -> p bl c", c=4),
                   selc[:].rearrange("p (bl c) -> p bl c", c=4),
                   selv, AL.mult)
            nc.sync.dma_start(out=loss_d[:, :], in_=SQ[:])

    nc.compile()
    return nc


_HOST_CONSTS = _host_consts()
_G24 = np.array([GRIDS[s // 2] for s in range(SBL)], np.float32)


def make_in_maps(output0, anchors0, output1, anchors1, output2, anchors2,
                 targets):
    outs = [np.asarray(output0), np.asarray(output1), np.asarray(output2)]
    ancs = [np.asarray(anchors0), np.asarray(anchors1), np.asarray(anchors2)]
    tg = np.asarray(targets)

    nawh = np.zeros(36, np.float32)
    awh = np.zeros(36, np.float32)
    areaa = np.zeros(18, np.float32)
    for s_ in range(3):
        for a_ in range(A):
            aw, ah = float(ancs[s_][a_, 0]), float(ancs[s_][a_, 1])
            for bl in range(2):
                sbl = s_ * 2 + bl
                for q_, dim in ((0, aw), (1, ah)):
                    nawh[q_ * 18 + sbl * 3 + a_] = -0.5 * dim
                    awh[q_ * 18 + sbl * 3 + a_] = 0.5 * dim
                areaa[sbl * 3 + a_] = aw * ah

    in_maps = []
    for c in range(NCORES):
        sl = slice(c * PB, (c + 1) * PB)
        raw = tg[sl, :, 1:5].astype(np.float32)          # [4, 50, 4]
        tg8 = (raw.reshape(2, 2, T, 4)                    # (bh, bl, j, c)
               .transpose(0, 2, 1, 3).reshape(P100, 2, 4))  # (bh,j) x bl x c
        t4 = (tg8[:, None, :, :] *
              _G24.reshape(1, 3, 2, 1)[:, :, :, [0, 0, 0, 0]])  # wrong shape fix below
        # t4[p, (s, bl), c] = raw * g(s)
        t4 = (tg8[:, None, :, :] * _G24.reshape(3, 2)[None, :, :, None]
              ).reshape(P100, 24)
        t4r = t4.reshape(P100, 6, 4)
        wh2 = np.concatenate([-0.5 * t4r[:, :, 2:4].reshape(P100, 12),
                              0.5 * t4r[:, :, 2:4].reshape(P100, 12)], axis=1)
        valid = (np.abs(raw).sum(2) > 0).astype(np.float32)  # [4, 50]
        val2 = (valid.reshape(2, 2, T).transpose(0, 2, 1)
                .reshape(P100, 2))                           # (bh,j) x bl
        hostpack = _HOST_CONSTS.copy()
        hostpack[:, _H_T4:_H_T4 + 24] = t4
        hostpack[:, _H_WH2:_H_WH2 + 24] = wh2
        hostpack[:, _H_VAL:_H_VAL + 2] = val2
        hostpack[:, _H_NAWH:_H_NAWH + 36] = nawh[None, :]
        hostpack[:, _H_AWH:_H_AWH + 36] = awh[None, :]
        hostpack[:, _H_AREAA:_H_AREAA + 18] = areaa[None, :]
        outcat = np.concatenate([o[sl].ravel() for o in outs]).astype(np.float32)
        in_maps.append({"hostpack": np.ascontiguousarray(hostpack),
                        "outcat": outcat})
    return in_maps


_NC_CACHE = {}


def kernel(output0, anchors0, output1, anchors1, output2, anchors2, targets):
    import time
    from concourse.bass_utils import run_bass_kernel_spmd

    if "nc" not in _NC_CACHE:
        _NC_CACHE["nc"] = build_nc(use_collective=False)
    nc = _NC_CACHE["nc"]
    in_maps = make_in_maps(output0, anchors0, output1, anchors1, output2,
                           anchors2, targets)
    res = None
    for attempt in range(3):
        try:
            res = run_bass_kernel_spmd(nc, in_maps, list(range(NCORES)))
            break
        except Exception:
            # transient NRT device errors have been observed; back off + retry
            if attempt == 2:
                raise
            time.sleep(20.0 * (attempt + 1))
    total = np.float64(0.0)
    for c in range(NCORES):
        total += np.asarray(res.results[c]["loss"], dtype=np.float64).sum()
    return np.float32(total)
